# revision 6
# baseline (speedup 1.0000x reference)
"""Trainium2 Bass kernel for nn_DecoderRNN (attention decoder step + vocab
log-softmax), distributed over 8 NeuronCores.

Sharding:
  - small phase (attn, combine, GRU): data-parallel over batch (32 rows/core)
  - output Linear + log_softmax: tensor-parallel over vocab (4000 cols/core)
  - collectives: AllGather of h_new (for the vocab-parallel matmul) and
    AllGather of per-core (rowmax, sumexp) stats for the global log-softmax.

All matmuls run in bf16 with fp32 PSUM accumulation; softmax/GRU gate math is
fp32. Weights are pre-transposed on the host during input staging; bias rows
are folded in via a K=1 ones-row matmul into the same PSUM accumulation.
"""
import contextlib
import sys

if "/opt/trn_rl_repo" not in sys.path:
    sys.path.insert(0, "/opt/trn_rl_repo")

import numpy as np
import ml_dtypes

import concourse.bacc as bacc
import concourse.mybir as mybir
from concourse import bass, masks, tile
from concourse.bass_utils import run_bass_kernel_spmd

F32 = mybir.dt.float32
BF16 = mybir.dt.bfloat16
AF = mybir.ActivationFunctionType
AX = mybir.AxisListType.X

N_CORES = 8
H = 1024          # hidden
V = 32000         # vocab
L = 512           # encoder positions
B = 256           # batch
BC = B // N_CORES         # batch rows per core (32)
VC = V // N_CORES         # vocab cols per core (4000)
NCH = 8                   # vocab chunks per core
CH = VC // NCH            # 500, fits one PSUM bank (f32)
KH = H // 128             # 8  K-tiles over hidden
KL = L // 128             # 4  K-tiles over encoder positions


def _pack_T(a: np.ndarray) -> np.ndarray:
    """[BC, D] -> [128, (D//128)*BC] bf16: column-block k of `a` transposed
    into columns [BC*k : BC*(k+1)].  Staged activation lhsT layout."""
    bc, d = a.shape
    k = d // 128
    t = a.reshape(bc, k, 128).transpose(2, 1, 0)  # [128, k, bc]
    return np.ascontiguousarray(t.reshape(128, k * bc).astype(ml_dtypes.bfloat16))


def build_nc():
    nc = bacc.Bacc(
        "TRN2",
        target_bir_lowering=False,
        debug=False,
        enable_asserts=True,
        num_devices=N_CORES,
    )

    # ---- per-core external inputs ----
    embT = nc.dram_tensor("embT", [128, KH * BC], BF16, kind="ExternalInput")
    hT = nc.dram_tensor("hT", [128, KH * BC], BF16, kind="ExternalInput")
    h_nat = nc.dram_tensor("h_nat", [BC, H], F32, kind="ExternalInput")
    attn_WT = nc.dram_tensor("attn_WT", [2 * H, L], BF16, kind="ExternalInput")
    attn_b_r = nc.dram_tensor("attn_b_r", [1, L], BF16, kind="ExternalInput")
    enc = nc.dram_tensor("enc", [L, H], BF16, kind="ExternalInput")
    comb_WT = nc.dram_tensor("comb_WT", [2 * H, H], BF16, kind="ExternalInput")
    comb_b_r = nc.dram_tensor("comb_b_r", [1, H], BF16, kind="ExternalInput")
    W_ihT = nc.dram_tensor("W_ihT", [H, 3 * H], BF16, kind="ExternalInput")
    b_ih_r = nc.dram_tensor("b_ih_r", [1, 3 * H], BF16, kind="ExternalInput")
    W_hhT = nc.dram_tensor("W_hhT", [H, 3 * H], BF16, kind="ExternalInput")
    b_hh_r = nc.dram_tensor("b_hh_r", [1, 3 * H], BF16, kind="ExternalInput")
    out_WT = nc.dram_tensor("out_WT", [H, VC], BF16, kind="ExternalInput")
    out_b_r = nc.dram_tensor("out_b_r", [1, VC], BF16, kind="ExternalInput")

    # ---- per-core external outputs ----
    out_log = nc.dram_tensor("out_log", [B, VC], F32, kind="ExternalOutput")
    h_new_o = nc.dram_tensor("h_new_o", [BC, H], F32, kind="ExternalOutput")
    attn_w_o = nc.dram_tensor("attn_w_o", [BC, L], F32, kind="ExternalOutput")

    rg = [list(range(N_CORES))]

    with tile.TileContext(nc) as tc, contextlib.ExitStack() as stack:
        const = stack.enter_context(tc.tile_pool(name="const", bufs=1))
        acts = stack.enter_context(tc.tile_pool(name="acts", bufs=1))
        wq = stack.enter_context(tc.tile_pool(name="wq", bufs=2))
        encp = stack.enter_context(tc.tile_pool(name="encp", bufs=4))
        gruq = stack.enter_context(tc.tile_pool(name="gruq", bufs=2))
        outwq = stack.enter_context(tc.tile_pool(name="outwq", bufs=8))
        ps = stack.enter_context(tc.tile_pool(name="ps", bufs=8, space="PSUM"))
        sm = stack.enter_context(tc.tile_pool(name="sm", bufs=2))
        lgp = stack.enter_context(tc.tile_pool(name="lgp", bufs=1))
        outfp = stack.enter_context(tc.tile_pool(name="outfp", bufs=1))
        dram = stack.enter_context(tc.tile_pool(name="dram", bufs=1, space="DRAM"))

        ident = const.tile([128, 128], BF16, tag="ident")
        masks.make_identity(nc, ident[:])
        ones = const.tile([1, 128], BF16, tag="ones")
        nc.gpsimd.memset(ones[:], 1.0)

        # resident staged activations
        embT_sb = const.tile([128, KH * BC], BF16, tag="embT")
        nc.sync.dma_start(embT_sb[:], embT[:])
        hT_sb = const.tile([128, KH * BC], BF16, tag="hT")
        nc.sync.dma_start(hT_sb[:], hT[:])
        h_nat_sb = const.tile([BC, H], F32, tag="h_nat")
        nc.sync.dma_start(h_nat_sb[:], h_nat[:])

        def load_row(t, n):
            r = const.tile([1, t.shape[1]], BF16, tag=n)
            nc.sync.dma_start(r[:], t[:])
            return r

        attn_b_sb = load_row(attn_b_r, "attn_b")
        comb_b_sb = load_row(comb_b_r, "comb_b")
        b_ih_sb = load_row(b_ih_r, "b_ih")
        b_hh_sb = load_row(b_hh_r, "b_hh")
        out_b_sb = load_row(out_b_r, "out_b")

        # out_W stream: issue loads now so they overlap the small phase
        outw_tiles = []
        for k in range(KH):
            t = outwq.tile([128, VC], BF16, tag="outw")
            nc.sync.dma_start(t[:], out_WT[128 * k:128 * (k + 1), :])
            outw_tiles.append(t)

        # ---------------- attention logits + softmax ----------------
        ps_al = ps.tile([BC, L], F32, tag="ps")
        for k in range(2 * KH):
            lhsT = (embT_sb if k < KH else hT_sb)[:, BC * (k % KH):BC * (k % KH + 1)]
            w = wq.tile([128, L], BF16, tag="attnw")
            nc.sync.dma_start(w[:], attn_WT[128 * k:128 * (k + 1), :])
            nc.tensor.matmul(ps_al[:], lhsT, w[:], start=(k == 0), stop=False)
        nc.tensor.matmul(ps_al[:], ones[:1, :BC], attn_b_sb[:], start=False, stop=True)

        neg_m = acts.tile([BC, 1], F32, tag="negm")
        nc.vector.reduce_max(out=neg_m[:], in_=ps_al[:], axis=AX, negate=True)
        ssum = acts.tile([BC, 1], F32, tag="ssum")
        aw_f = acts.tile([BC, L], F32, tag="awf")
        nc.scalar.activation(aw_f[:], ps_al[:], AF.Exp, bias=neg_m[:], accum_out=ssum[:])
        rinv = acts.tile([BC, 1], F32, tag="rinv")
        nc.vector.reciprocal(rinv[:], ssum[:])
        nc.vector.tensor_scalar_mul(aw_f[:], aw_f[:], rinv[:])
        nc.sync.dma_start(attn_w_o[:], aw_f[:])
        aw_b = acts.tile([BC, L], BF16, tag="awb")
        nc.vector.tensor_copy(aw_b[:], aw_f[:])

        # transpose attn weights: awT [128, KL*BC]
        awT = acts.tile([128, KL * BC], BF16, tag="awT")
        for k in range(KL):
            pt = ps.tile([128, BC], BF16, tag="ps")
            nc.tensor.transpose(pt[:], aw_b[:, 128 * k:128 * (k + 1)], ident[:BC, :BC])
            nc.vector.tensor_copy(awT[:, BC * k:BC * (k + 1)], pt[:])

        # encoder tiles (resident through attn_applied)
        enc_tiles = []
        for k in range(KL):
            t = encp.tile([128, H], BF16, tag="enc")
            nc.sync.dma_start(t[:], enc[128 * k:128 * (k + 1), :])
            enc_tiles.append(t)

        # ---------------- attn_applied = aw @ enc  (natural [BC, H]) ----
        aa_b = acts.tile([BC, H], BF16, tag="aab")
        for c in range(2):
            pa = ps.tile([BC, 512], F32, tag="ps")
            for k in range(KL):
                nc.tensor.matmul(
                    pa[:], awT[:, BC * k:BC * (k + 1)],
                    enc_tiles[k][:, 512 * c:512 * (c + 1)],
                    start=(k == 0), stop=(k == KL - 1),
                )
            nc.scalar.activation(aa_b[:, 512 * c:512 * (c + 1)], pa[:], AF.Copy)

        # transpose attn_applied: aaT [128, KH*BC]
        aaT = acts.tile([128, KH * BC], BF16, tag="aaT")
        for k in range(KH):
            pt = ps.tile([128, BC], BF16, tag="ps")
            nc.tensor.transpose(pt[:], aa_b[:, 128 * k:128 * (k + 1)], ident[:BC, :BC])
            nc.vector.tensor_copy(aaT[:, BC * k:BC * (k + 1)], pt[:])

        # ---------------- x = relu([emb; aa] @ comb_W.T + b)  ([BC, H]) --
        x_b = acts.tile([BC, H], BF16, tag="xb")
        comb_tiles = []
        for k in range(2 * KH):
            w = wq.tile([128, H], BF16, tag="combw")
            nc.sync.dma_start(w[:], comb_WT[128 * k:128 * (k + 1), :])
            comb_tiles.append(w)
        for c in range(2):
            px = ps.tile([BC, 512], F32, tag="ps")
            for k in range(2 * KH):
                lhsT = (embT_sb if k < KH else aaT)[:, BC * (k % KH):BC * (k % KH + 1)]
                nc.tensor.matmul(
                    px[:], lhsT, comb_tiles[k][:, 512 * c:512 * (c + 1)],
                    start=(k == 0), stop=False,
                )
            nc.tensor.matmul(
                px[:], ones[:1, :BC], comb_b_sb[:, 512 * c:512 * (c + 1)],
                start=False, stop=True,
            )
            nc.scalar.activation(x_b[:, 512 * c:512 * (c + 1)], px[:], AF.Relu)

        # transpose x: xT [128, KH*BC]
        xT = acts.tile([128, KH * BC], BF16, tag="xT")
        for k in range(KH):
            pt = ps.tile([128, BC], BF16, tag="ps")
            nc.tensor.transpose(pt[:], x_b[:, 128 * k:128 * (k + 1)], ident[:BC, :BC])
            nc.vector.tensor_copy(xT[:, BC * k:BC * (k + 1)], pt[:])

        # ---------------- GRU gates gi, gh  (natural [BC, 3H], bf16) ----
        def gemm_3h(lhsT_pack, w_dram, b_sb, name):
            g_sb = acts.tile([BC, 3 * H], BF16, tag=name)
            w_tiles = []
            for k in range(KH):
                w = gruq.tile([128, 3 * H], BF16, tag="gruw")
                nc.sync.dma_start(w[:], w_dram[128 * k:128 * (k + 1), :])
                w_tiles.append(w)
            for c in range(6):
                pg = ps.tile([BC, 512], F32, tag="ps")
                for k in range(KH):
                    nc.tensor.matmul(
                        pg[:], lhsT_pack[:, BC * k:BC * (k + 1)],
                        w_tiles[k][:, 512 * c:512 * (c + 1)],
                        start=(k == 0), stop=False,
                    )
                nc.tensor.matmul(
                    pg[:], ones[:1, :BC], b_sb[:, 512 * c:512 * (c + 1)],
                    start=False, stop=True,
                )
                nc.scalar.activation(g_sb[:, 512 * c:512 * (c + 1)], pg[:], AF.Copy)
            return g_sb

        gi = gemm_3h(xT, W_ihT, b_ih_sb, "gi")
        gh = gemm_3h(hT_sb, W_hhT, b_hh_sb, "gh")

        # gates (fp32 math, natural [BC, H] slices, 3 working buffers)
        tmp = acts.tile([BC, H], F32, tag="gtmp")
        rn_g = acts.tile([BC, H], F32, tag="rng")   # holds r, then n
        z_g = acts.tile([BC, H], F32, tag="zg")
        nc.vector.tensor_add(tmp[:], gi[:, 0:H], gh[:, 0:H])
        nc.scalar.activation(rn_g[:], tmp[:], AF.Sigmoid)          # r
        nc.vector.tensor_add(tmp[:], gi[:, H:2 * H], gh[:, H:2 * H])
        nc.scalar.activation(z_g[:], tmp[:], AF.Sigmoid)           # z
        nc.vector.tensor_mul(tmp[:], rn_g[:], gh[:, 2 * H:3 * H])  # r*gh_n
        nc.vector.tensor_add(tmp[:], tmp[:], gi[:, 2 * H:3 * H])
        nc.scalar.activation(rn_g[:], tmp[:], AF.Tanh)             # n
        # h_new = n + z*(h - n)
        h_new = acts.tile([BC, H], F32, tag="hnew")
        nc.vector.tensor_sub(tmp[:], h_nat_sb[:], rn_g[:])
        nc.vector.tensor_mul(tmp[:], z_g[:], tmp[:])
        nc.vector.tensor_add(h_new[:], rn_g[:], tmp[:])
        nc.sync.dma_start(h_new_o[:], h_new[:])
        h_new_b = acts.tile([BC, H], BF16, tag="hnewb")
        nc.vector.tensor_copy(h_new_b[:], h_new[:])

        # ---------------- AllGather h_new ----------------
        hg_in = dram.tile([BC, H], BF16, tag="hg_in")
        hg_out = dram.tile([B, H], BF16, tag="hg_out")
        nc.gpsimd.dma_start(hg_in[:], h_new_b[:])
        nc.gpsimd.collective_compute(
            "AllGather", mybir.AluOpType.bypass,
            ins=[hg_in.opt()], outs=[hg_out.opt()], replica_groups=rg,
        )

        # load gathered h_new and transpose into lhsT tiles [128, B]
        hfT = [acts.tile([128, B], BF16, tag=f"hfT{k}", name=f"hfT{k}") for k in range(KH)]
        for bt in range(2):
            hg_sb = sm.tile([128, H], BF16, tag="hg")
            nc.gpsimd.dma_start(hg_sb[:], hg_out[128 * bt:128 * (bt + 1), :])
            for k in range(KH):
                pt = ps.tile([128, 128], BF16, tag="ps")
                nc.tensor.transpose(pt[:], hg_sb[:, 128 * k:128 * (k + 1)], ident[:])
                nc.vector.tensor_copy(hfT[k][:, 128 * bt:128 * (bt + 1)], pt[:])

        # ---------------- big matmul: logits = h_new @ out_W.T + b ------
        # bt=0: spill logits to SBUF (bf16); bt=1: keep in PSUM through the
        # stats collective and write the output directly from PSUM.
        logits0 = lgp.tile([128, VC], BF16, tag="logits0")
        ms_sb = []
        bt1_banks = None
        for bt in range(2):
            pbanks = [ps.tile([128, CH], F32, tag="ps", name=f"pbank{bt}_{c}") for c in range(NCH)]
            for k in range(KH):
                lhsT = hfT[k][:, 128 * bt:128 * (bt + 1)]
                for c in range(NCH):
                    nc.tensor.matmul(
                        pbanks[c][:], lhsT,
                        outw_tiles[k][:, CH * c:CH * (c + 1)],
                        start=(k == 0), stop=False,
                    )
            for c in range(NCH):
                nc.tensor.matmul(
                    pbanks[c][:], ones[:1, :128],
                    out_b_sb[:, CH * c:CH * (c + 1)],
                    start=False, stop=True,
                )
            mparts = acts.tile([128, NCH], F32, tag="mparts")
            for c in range(NCH):
                if bt == 0:
                    nc.vector.tensor_copy(logits0[:, CH * c:CH * (c + 1)], pbanks[c][:])
                nc.vector.reduce_max(out=mparts[:, c:c + 1], in_=pbanks[c][:], axis=AX)
            m_loc = acts.tile([128, 1], F32, tag="mloc")
            nc.vector.reduce_max(out=m_loc[:], in_=mparts[:], axis=AX)
            neg_ml = acts.tile([128, 1], F32, tag="negml")
            nc.scalar.mul(neg_ml[:], m_loc[:], -1.0)
            sparts = acts.tile([128, NCH], F32, tag="sparts")
            for c in range(NCH):
                ej = sm.tile([128, CH], BF16, tag="ejunk")
                nc.scalar.activation(
                    ej[:], pbanks[c][:], AF.Exp, bias=neg_ml[:],
                    accum_out=sparts[:, c:c + 1],
                )
            s_loc = acts.tile([128, 1], F32, tag="sloc")
            nc.vector.reduce_sum(out=s_loc[:], in_=sparts[:], axis=AX)
            ms = acts.tile([128, 2], F32, tag=f"ms{bt}")
            nc.vector.tensor_copy(ms[:, 0:1], m_loc[:])
            nc.vector.tensor_copy(ms[:, 1:2], s_loc[:])
            ms_sb.append(ms)
            if bt == 1:
                bt1_banks = pbanks

        # ---------------- AllGather (m, s) stats ----------------
        ms_in = dram.tile([B, 2], F32, tag="ms_in")
        ms_out = dram.tile([N_CORES * B, 2], F32, tag="ms_out")
        for bt in range(2):
            nc.gpsimd.dma_start(ms_in[128 * bt:128 * (bt + 1), :], ms_sb[bt][:])
        nc.gpsimd.collective_compute(
            "AllGather", mybir.AluOpType.bypass,
            ins=[ms_in.opt()], outs=[ms_out.opt()], replica_groups=rg,
        )

        # global logsumexp + final output
        ms_view = ms_out[:].rearrange("(r bt p) c -> bt p c r", r=N_CORES, bt=2, p=128)
        for bt in range(2):
            msg = acts.tile([128, 2 * N_CORES], F32, tag="msg")
            nc.gpsimd.dma_start(msg[:].rearrange("p (c r) -> p c r", c=2, r=N_CORES), ms_view[bt])
            gm = acts.tile([128, 1], F32, tag="gm")
            nc.vector.reduce_max(out=gm[:], in_=msg[:, 0:N_CORES], axis=AX)
            neg_gm = acts.tile([128, 1], F32, tag="neggm")
            nc.scalar.mul(neg_gm[:], gm[:], -1.0)
            ew = acts.tile([128, N_CORES], F32, tag="ew")
            nc.scalar.activation(ew[:], msg[:, 0:N_CORES], AF.Exp, bias=neg_gm[:])
            nc.vector.tensor_mul(ew[:], ew[:], msg[:, N_CORES:2 * N_CORES])
            s_all = acts.tile([128, 1], F32, tag="sall")
            nc.vector.reduce_sum(out=s_all[:], in_=ew[:], axis=AX)
            log_s = acts.tile([128, 1], F32, tag="logs")
            nc.scalar.activation(log_s[:], s_all[:], AF.Ln)
            neg_lse = acts.tile([128, 1], F32, tag="neglse")
            nc.vector.tensor_add(neg_lse[:], gm[:], log_s[:])
            nc.scalar.mul(neg_lse[:], neg_lse[:], -1.0)
            outf = outfp.tile([128, VC], F32, tag="outf")
            if bt == 0:
                nc.scalar.activation(outf[:], logits0[:], AF.Identity, bias=neg_lse[:])
            else:
                for c in range(NCH):
                    nc.scalar.activation(
                        outf[:, CH * c:CH * (c + 1)], bt1_banks[c][:],
                        AF.Identity, bias=neg_lse[:],
                    )
            nc.sync.dma_start(out_log[128 * bt:128 * (bt + 1), :], outf[:])

    nc.compile()
    return nc


_NC_CACHE = None


def _get_nc():
    global _NC_CACHE
    if _NC_CACHE is None:
        _NC_CACHE = build_nc()
    return _NC_CACHE


def make_in_maps(inputs):
    inp = np.asarray(inputs["input"]).astype(np.int64)
    hidden = np.asarray(inputs["hidden"], np.float32)
    enc_np = np.asarray(inputs["encoder_outputs"], np.float32)
    emb = np.asarray(inputs["emb"], np.float32)

    h = hidden[0]                       # [B, H]
    embedded = emb[inp]                 # [B, H]

    bf = ml_dtypes.bfloat16

    def c(a, dt=bf):
        return np.ascontiguousarray(np.asarray(a, np.float32).astype(dt))

    shared = {
        "attn_WT": c(np.asarray(inputs["attn_W"]).T),
        "attn_b_r": c(np.asarray(inputs["attn_b"])[None, :]),
        "enc": c(enc_np),
        "comb_WT": c(np.asarray(inputs["comb_W"]).T),
        "comb_b_r": c(np.asarray(inputs["comb_b"])[None, :]),
        "W_ihT": c(np.asarray(inputs["W_ih"]).T),
        "b_ih_r": c(np.asarray(inputs["b_ih"])[None, :]),
        "W_hhT": c(np.asarray(inputs["W_hh"]).T),
        "b_hh_r": c(np.asarray(inputs["b_hh"])[None, :]),
    }
    out_WT_full = np.asarray(inputs["out_W"], np.float32).T
    out_b_full = np.asarray(inputs["out_b"], np.float32)
    in_maps = []
    for j in range(N_CORES):
        rows = slice(BC * j, BC * (j + 1))
        cols = slice(VC * j, VC * (j + 1))
        m = dict(shared)
        m["embT"] = _pack_T(embedded[rows])
        m["hT"] = _pack_T(h[rows])
        m["h_nat"] = c(h[rows], np.float32)
        m["out_WT"] = c(out_WT_full[:, cols])
        m["out_b_r"] = c(out_b_full[None, cols])
        in_maps.append(m)
    return in_maps


def run_device(inputs, trace=False):
    nc = _get_nc()
    in_maps = make_in_maps(inputs)
    res = run_bass_kernel_spmd(nc, in_maps, list(range(N_CORES)), trace=trace)
    out = np.empty((B, V), np.float32)
    h_new = np.empty((B, H), np.float32)
    attn_w = np.empty((B, L), np.float32)
    for j in range(N_CORES):
        r = res.results[j]
        out[:, VC * j:VC * (j + 1)] = r["out_log"]
        h_new[BC * j:BC * (j + 1)] = r["h_new_o"]
        attn_w[BC * j:BC * (j + 1)] = r["attn_w_o"]
    return (out, h_new[None], attn_w), res


def kernel(**inputs):
    (out, h_new, attn_w), _ = run_device(inputs, trace=False)
    return out, h_new, attn_w
